# revision 4
# baseline (speedup 1.0000x reference)
"""Trainium2 Bass kernel for multi-head attention (B=4, N=2048, DIM=768, H=12)
with host-side mask compaction via token permutation, bf16 operands.

Sharding: 8 cores; core c handles batch b = c//2 and heads h0 = 6*(c%2)..+5.
Each core emits a partial projection output (bf16, token-permuted); the host
unpermutes rows and sums the two partials per batch.

Mask compaction: the host permutes the token axis so unmasked tokens come
first (m of them, m <= NT_K*128). One x tensor serves both paths: K/V are
computed only over the first NT_K*128 permuted tokens (positions >= m are
masked tokens and get -80 exp bias, i.e. weight ~3e-35 ~ 0, matching the
reference's -inf within ~1e-30); Q runs over all 2048 permuted tokens and the
output rows are unpermuted host-side. This cuts S, exp and PV work by ~7/16
and sends x once.

All matmul operands are bf16 (PSUM accumulation stays fp32); biases and the
exp mask bias are fp32. Device pipeline per (q-block, head-pair): S^T = K@Q^T
per k-tile in PSUM -> exp with per-partition bias + 1/8 scale in one ACT op
over [128, 1024] (2 heads) -> accumulating P^T@V matmuls with a per-head ones
column producing the softmax denominator in row 64 -> reciprocal straight
from PSUM + gpsimd partition-broadcast + multiply into the attnT tile ->
proj matmuls with bias added during DVE eviction. QKV/V/proj matmul groups
stream as fillers inside the attention iterations.
"""

import numpy as np
from contextlib import ExitStack

import concourse.bass as bass
import concourse.tile as tile
from concourse import bacc, mybir
from concourse.alu_op_type import AluOpType
from concourse.bass_utils import run_bass_kernel_spmd

N_CORES = 8
B, N, DIM = 4, 2048, 768
NHEADS, HD = 12, 64
HPC = 6              # heads per core
DPC = HPC * HD       # 384 channels per core
CC = DIM // 128      # 6 contraction chunks of 128
NT_K_COMPACT = 9     # compacted key tiles (covers up to 1152 unmasked keys)
QB = 512             # q block width
NQB = N // QB        # 4 q blocks
KB = 384             # k-gen eviction block width
VW = HPC * (HD + 1)  # V width incl per-head ones column (390)
BIG = 80.0
F32 = mybir.dt.float32
BF16 = mybir.dt.bfloat16
AF = mybir.ActivationFunctionType
NPBF16 = mybir.dt.np(BF16)

_CACHE = {}


def build(nt_k=NT_K_COMPACT, reps=1):
    NK = nt_k * 128          # compacted key width
    NKB = NK // KB           # k-gen blocks
    assert NK % KB == 0

    nc = bacc.Bacc("TRN2", target_bir_lowering=False, debug=False,
                   num_devices=N_CORES)

    xT = nc.dram_tensor("xT", [DIM, N], BF16, kind="ExternalInput").ap()
    wqT = nc.dram_tensor("wqT", [DIM, DPC], BF16, kind="ExternalInput").ap()
    wkT = nc.dram_tensor("wkT", [DIM, DPC], BF16, kind="ExternalInput").ap()
    wvT = nc.dram_tensor("wvT", [DIM, VW], BF16, kind="ExternalInput").ap()
    qbias = nc.dram_tensor("qbias", [128, 3], F32, kind="ExternalInput").ap()
    mbias = nc.dram_tensor("mbias", [128, nt_k], F32, kind="ExternalInput").ap()
    pwT = nc.dram_tensor("pwT", [DPC, DIM], BF16, kind="ExternalInput").ap()
    pbb = nc.dram_tensor("pbb", [128, DIM], F32, kind="ExternalInput").ap()
    vbb = nc.dram_tensor("vbb", [128, VW], F32, kind="ExternalInput").ap()
    out = nc.dram_tensor("out", [N, DIM], BF16, kind="ExternalOutput").ap()

    with tile.TileContext(nc) as tc, ExitStack() as ctx:
        # PSUM (8 banks): tag "s" [128,1024]x2 = 4, tag "o" [65,512]x2 = 2,
        # tag "pj" [128,<=512]x2 = 2 (QKV/V/proj rotation).
        psum = ctx.enter_context(tc.tile_pool(name="psum", bufs=2, space="PSUM"))
        persist = ctx.enter_context(tc.tile_pool(name="persist", bufs=1))
        qt_sb = [persist.tile([128, N], BF16, tag=f"qt{j}", name=f"qt{j}")
                 for j in range(3)]
        # K stored twice, zero-padded to 128 contraction rows per head, so S
        # matmuls stay in 128-row array mode (no 64<->128 mode-switch drains
        # against the 128-mode PV/QKV/proj stream): ktp[j][0] has head 2j in
        # rows 0:64 and zeros in 64:128, ktp[j][1] the reverse.
        ktp = [[persist.tile([128, NK], BF16, tag=f"ktp{j}_{h}",
                             name=f"ktp{j}_{h}") for h in range(2)]
               for j in range(3)]
        v_sb = [persist.tile([128, VW], BF16, tag=f"v{t}", name=f"v{t}")
                for t in range(nt_k)]
        consts = ctx.enter_context(tc.tile_pool(name="consts", bufs=1))
        qb_sb = consts.tile([128, 3], F32, tag="qb", name="qb")
        mb_sb = consts.tile([128, nt_k], F32, tag="mb", name="mb")
        pbb_sb = consts.tile([128, DIM], F32, tag="pbb", name="pbb")
        vbb_sb = consts.tile([128, VW], F32, tag="vbb", name="vbb")
        pw_sb = [consts.tile([128, DIM], BF16, tag=f"pw{j}", name=f"pw{j}")
                 for j in range(3)]
        xw = ctx.enter_context(tc.tile_pool(name="xw", bufs=1))
        x_sb = [xw.tile([128, N], BF16, tag=f"x{c}", name=f"x{c}")
                for c in range(CC)]
        wq_sb = [xw.tile([128, DPC], BF16, tag=f"wq{c}", name=f"wq{c}")
                 for c in range(CC)]
        wk_sb = [xw.tile([128, DPC], BF16, tag=f"wk{c}", name=f"wk{c}")
                 for c in range(CC)]
        wv_sb = [xw.tile([128, VW], BF16, tag=f"wv{c}", name=f"wv{c}")
                 for c in range(CC)]
        ppool = ctx.enter_context(tc.tile_pool(name="ppool", bufs=6))
        atpool = ctx.enter_context(tc.tile_pool(name="atpool", bufs=2))
        dpool = ctx.enter_context(tc.tile_pool(name="dpool", bufs=2))
        opool = ctx.enter_context(tc.tile_pool(name="opool", bufs=2))

        # ---- input DMAs. Tiny consts first (FIFO queue: they'd otherwise
        # wait behind megabytes), then the K-region columns of each x chunk
        # (all the prologue needs), then the q-only columns and late consts.
        nc.sync.dma_start(qb_sb[:], qbias)
        nc.sync.dma_start(mb_sb[:], mbias)
        nc.sync.dma_start(vbb_sb[:], vbb)
        for c in range(CC):
            csl = slice(c * 128, (c + 1) * 128)
            nc.sync.dma_start(x_sb[c][:, 0:NK], xT[csl, 0:NK])
            nc.sync.dma_start(wk_sb[c][:], wkT[csl, :])
            nc.sync.dma_start(wq_sb[c][:], wqT[csl, :])
            nc.sync.dma_start(wv_sb[c][:], wvT[csl, :])
        for c in range(CC):
            nc.sync.dma_start(x_sb[c][:, NK:N], xT[c * 128:(c + 1) * 128, NK:N])
        nc.sync.dma_start(pbb_sb[:], pbb)
        for j in range(3):
            nc.sync.dma_start(pw_sb[j][:], pwT[j * 128:(j + 1) * 128, :])

        _q_open = {}

        def emit_q_half(j, nb, half):
            # half of a [128, 512] Q chunk (3 contraction steps each)
            dsl = bass.ts(j, 128)
            nsl = bass.ts(nb, QB)
            if half == 0:
                ps = psum.tile([128, QB], F32, tag="pj", name=f"q{j}_{nb}")
                _q_open[(j, nb)] = ps
            else:
                ps = _q_open.pop((j, nb))
            for c in range(3 * half, 3 * half + 3):
                nc.tensor.matmul(ps[:], wq_sb[c][:, dsl], x_sb[c][:, nsl],
                                 start=(c == 0), stop=(c == CC - 1))
            if half == 1:
                nc.vector.tensor_scalar(qt_sb[j][:, nsl], ps[:],
                                        qb_sb[:, j:j + 1], None,
                                        op0=AluOpType.add)

        def emit_q_group(j, nb):
            emit_q_half(j, nb, 0)
            emit_q_half(j, nb, 1)

        # zero the dead halves of the padded K tiles once
        for j in range(3):
            nc.vector.memset(ktp[j][0][64:128, :], 0)
            nc.vector.memset(ktp[j][1][0:64, :], 0)

        def emit_k_group(j, kb):
            # [128, KB] K chunk, all 6 contraction steps
            dsl = bass.ts(j, 128)
            nsl = bass.ts(kb, KB)
            ps = psum.tile([128, KB], F32, tag="pj", name=f"k{j}_{kb}")
            for c in range(CC):
                nc.tensor.matmul(ps[:], wk_sb[c][:, dsl], x_sb[c][:, nsl],
                                 start=(c == 0), stop=(c == CC - 1))
            nc.vector.tensor_copy(ktp[j][0][0:64, nsl], ps[0:64, :])
            nc.vector.tensor_copy(ktp[j][1][64:128, nsl], ps[64:128, :])

        def emit_v(t):
            tsl = bass.ts(t, 128)
            psv = psum.tile([128, VW], F32, tag="pj", name=f"v{t}")
            for c in range(CC):
                nc.tensor.matmul(psv[:], x_sb[c][:, tsl], wv_sb[c][:],
                                 start=(c == 0), stop=(c == CC - 1))
            nc.vector.tensor_tensor(v_sb[t][:], psv[:], vbb_sb[:],
                                    op=AluOpType.add)

        _pending_div = []

        def emit_attn(qb, j, fillers=()):
            # heads 2j, 2j+1 over q block qb: S^T -> exp -> P^T V -> divide
            fillers = list(fillers)
            qsl = bass.ds(qb * QB, QB)
            at_t = atpool.tile([128, QB], BF16, tag=f"at{j}", name=f"at{qb}_{j}")
            o_ps = [psum.tile([HD + 1, QB], F32, tag="o", name=f"o{qb}_{j}_{i}")
                    for i in range(2)]
            s_tiles = {}

            def emit_s(kt):
                ksl = bass.ts(kt, 128)
                s = psum.tile([128, 2 * QB], F32, tag="s",
                              name=f"s{qb}_{j}_{kt}")
                for hh in range(2):
                    nc.tensor.matmul(s[:, bass.ts(hh, QB)],
                                     ktp[j][hh][:, ksl], qt_sb[j][:, qsl],
                                     start=True, stop=True)
                s_tiles[kt] = s

            emit_s(0)
            for d in _pending_div:
                d()
            _pending_div.clear()
            for kt in range(nt_k):
                if kt + 1 < nt_k:
                    emit_s(kt + 1)   # one iteration ahead keeps ACT fed
                s = s_tiles.pop(kt)
                p = ppool.tile([128, 2 * QB], BF16, tag="p",
                               name=f"p{qb}_{j}_{kt}")
                nc.scalar.activation(p[:], s[:], AF.Exp,
                                     bias=mb_sb[:, kt:kt + 1], scale=0.125)
                for hh in range(2):
                    vsl = bass.ds((2 * j + hh) * (HD + 1), HD + 1)
                    nc.tensor.matmul(o_ps[hh], v_sb[kt][:, vsl],
                                     p[:, bass.ts(hh, QB)],
                                     start=(kt == 0), stop=(kt == nt_k - 1))
                if fillers:
                    fillers.pop(0)()
            for f in fillers:
                f()

            def _division():
                for hh in range(2):
                    rd = dpool.tile([1, QB], F32, tag="rd", name="rd")
                    bc = dpool.tile([64, QB], F32, tag="bc", name="bc")
                    nc.vector.reciprocal(rd[:], o_ps[hh][HD:HD + 1, :])
                    nc.gpsimd.partition_broadcast(bc[:], rd[:])
                    nc.vector.tensor_tensor(at_t[bass.ds(hh * 64, 64), :],
                                            o_ps[hh][0:HD, :], bc[:],
                                            op=AluOpType.mult)
            _pending_div.append(_division)
            return at_t

        def emit_proj_tile(t, qb, at_tiles):
            tsl = bass.ts(t, 128)
            po = [psum.tile([128, 384], F32, tag="pj", name=f"po{t}_{e}")
                  for e in range(2)]
            for e in range(2):
                esl = bass.ts(e, 384)
                for j in range(3):
                    nc.tensor.matmul(po[e],
                                     at_tiles[j][:, bass.ts(t - 4 * qb, 128)],
                                     pw_sb[j][:, esl],
                                     start=(j == 0), stop=(j == 2))
            for e in range(2):
                ot = opool.tile([128, 384], BF16, tag="ot", name=f"ot{t}_{e}")
                nc.vector.tensor_tensor(ot[:], po[e],
                                        pbb_sb[:, bass.ts(e, 384)],
                                        op=AluOpType.add)
                nc.sync.dma_start(out[tsl, bass.ts(e, 384)], ot[:])

        # ---- emission: stream QKV/proj groups into the attention iterations.
        # Fillers spread so later (ACT-bound) calls each get ~2 PE units;
        # proj(qb) runs up to two q-blocks late (atpool bufs=2 keeps the attnT
        # tiles of two generations alive).
        import functools

        def P(f, *a):
            return functools.partial(f, *a)

        rep_ctx = tc.For_i(0, reps, 1) if reps > 1 else None
        if rep_ctx is not None:
            rep_ctx.__enter__()
        for _rep in range(1):
            # minimal serial prologue: K0 all blocks, Q0 block 0, V0/V1
            for kb in range(NKB):
                emit_k_group(0, kb)
            emit_q_group(0, 0)
            emit_v(0)
            emit_v(1)
            at_q = [None] * NQB
            fills = {
                (0, 0): [P(emit_v, t) for t in range(2, nt_k)] +
                        [P(emit_k_group, 1, kb) for kb in range(NKB - 1)],
                (0, 1): [P(emit_k_group, 2, kb) for kb in range(NKB)] +
                        [P(emit_q_group, 0, 1)],
                (0, 2): [P(emit_q_group, 1, 1), P(emit_q_group, 2, 1)],
            }

            def pj(qb, t):
                return P(emit_proj_tile, t, qb, at_q[qb])

            ats = [emit_attn(0, 0, fills[(0, 0)])]
            emit_k_group(1, NKB - 1)
            emit_q_group(1, 0)
            ats.append(emit_attn(0, 1, fills[(0, 1)]))
            emit_q_group(2, 0)
            ats.append(emit_attn(0, 2, fills[(0, 2)]))
            at_q[0] = ats

            ats = [emit_attn(1, 0, [pj(0, 0), P(emit_q_group, 0, 2)]),
                   emit_attn(1, 1, [pj(0, 1), P(emit_q_group, 1, 2)]),
                   emit_attn(1, 2, [pj(0, 2), P(emit_q_group, 2, 2)])]
            at_q[1] = ats
            ats = [emit_attn(2, 0, [pj(0, 3), P(emit_q_group, 0, 3)]),
                   emit_attn(2, 1, [pj(1, 4), P(emit_q_group, 1, 3)]),
                   emit_attn(2, 2, [pj(1, 5), P(emit_q_group, 2, 3)])]
            at_q[2] = ats
            ats = [emit_attn(3, 0, [pj(1, 6), pj(1, 7)]),
                   emit_attn(3, 1, [pj(2, 8), pj(2, 9)]),
                   emit_attn(3, 2, [pj(2, 10), pj(2, 11)])]
            at_q[3] = ats
            for d in _pending_div:
                d()
            _pending_div.clear()
            for t in range(12, 16):
                emit_proj_tile(t, 3, at_q[3])
        if rep_ctx is not None:
            rep_ctx.__exit__(None, None, None)

    nc.compile()
    return nc


def _prep_inputs(x, attention_mask, qkv_w, q_bias, v_bias, proj_w, proj_b,
                 nt_k=NT_K_COMPACT):
    NK = nt_k * 128
    in_maps = []
    perms = []
    f32 = np.float32
    x = np.asarray(x, f32)
    attention_mask = np.asarray(attention_mask)
    qkv_w = np.asarray(qkv_w, f32)
    for c in range(N_CORES):
        b, h0 = c // 2, (c % 2) * HPC
        rs = slice(h0 * HD, h0 * HD + DPC)
        cols = np.flatnonzero(attention_mask[b])
        m = len(cols)
        assert m <= NK
        perm = np.concatenate([cols, np.flatnonzero(attention_mask[b] == 0)])
        perms.append(perm)
        xT = np.ascontiguousarray(x[b][perm].T)
        mb = np.full(NK, -BIG, f32)
        mb[:m] = 0.0
        mb = np.ascontiguousarray(mb.reshape(nt_k, 128).T)

        wqT = np.ascontiguousarray(qkv_w[rs, :].T)
        wkT = np.ascontiguousarray(
            qkv_w[DIM + h0 * HD: DIM + h0 * HD + DPC, :].T)
        wvT = np.zeros((DIM, VW), f32)
        for h in range(HPC):
            wr = qkv_w[2 * DIM + (h0 + h) * HD: 2 * DIM + (h0 + h) * HD + HD, :]
            wvT[:, h * (HD + 1): h * (HD + 1) + HD] = wr.T

        qb = np.ascontiguousarray(np.asarray(q_bias, f32)[rs].reshape(3, 128).T)
        pwT = np.ascontiguousarray(np.asarray(proj_w, f32)[:, rs].T)
        pb = np.asarray(proj_b, f32) if c % 2 == 0 else np.zeros(DIM, f32)
        pbb = np.ascontiguousarray(np.broadcast_to(pb, (128, DIM)), f32)
        vb_row = np.zeros(VW, f32)
        for h in range(HPC):
            vb_row[h * (HD + 1): h * (HD + 1) + HD] = \
                np.asarray(v_bias, f32)[(h0 + h) * HD: (h0 + h + 1) * HD]
            vb_row[h * (HD + 1) + HD] = 1.0
        vbb = np.ascontiguousarray(np.broadcast_to(vb_row, (128, VW)), f32)
        in_maps.append({
            "xT": xT.astype(NPBF16),
            "wqT": wqT.astype(NPBF16), "wkT": wkT.astype(NPBF16),
            "wvT": wvT.astype(NPBF16),
            "qbias": qb.astype(f32), "mbias": mb.astype(f32),
            "pwT": pwT.astype(NPBF16),
            "pbb": pbb, "vbb": vbb,
        })
    return in_maps, perms


def kernel(x, attention_mask, qkv_w, q_bias, v_bias, proj_w, proj_b):
    counts = np.asarray(attention_mask).astype(np.int64).sum(axis=1)
    nt_k = NT_K_COMPACT if counts.max() <= NT_K_COMPACT * 128 \
        else (int(counts.max()) + 127) // 128
    key = f"nc{nt_k}"
    if key not in _CACHE:
        _CACHE[key] = build(nt_k=nt_k)
    nc = _CACHE[key]
    in_maps, perms = _prep_inputs(x, attention_mask, qkv_w, q_bias, v_bias,
                                  proj_w, proj_b, nt_k=nt_k)
    res = run_bass_kernel_spmd(nc, in_maps, core_ids=list(range(N_CORES)))
    out = np.empty((B, N, DIM), np.float32)
    for b in range(B):
        dev = res.results[2 * b]["out"].astype(np.float32) \
            + res.results[2 * b + 1]["out"].astype(np.float32)
        out[b][perms[2 * b]] = dev
    return out


if __name__ == "__main__":
    import reference
    inputs = {k: np.asarray(v) for k, v in reference.setup_inputs().items()}
    got = kernel(**inputs)
    exp = np.asarray(reference.reference(**inputs))
    err = np.abs(got - exp).max()
    rel = err / np.abs(exp).max()
    print("max abs err:", err, "rel:", rel)


# revision 7
# speedup vs baseline: 1.1773x; 1.1773x over previous
"""Trainium2 Bass kernel for multi-head attention (B=4, N=2048, DIM=768, H=12)
with host-side mask compaction via token permutation, bf16 operands.

Sharding: 8 cores; core c handles batch b = c//2 and heads h0 = 6*(c%2)..+5.
Each core emits a partial projection output (bf16, token-permuted); the host
unpermutes rows and sums the two partials per batch.

Mask compaction: the host permutes the token axis so unmasked tokens come
first (m of them, m <= NT_K*128). One x tensor serves both paths: K/V are
computed only over the first NT_K*128 permuted tokens (positions >= m are
masked tokens and get -80 exp bias, i.e. weight ~3e-35 ~ 0, matching the
reference's -inf within ~1e-30); Q runs over all 2048 permuted tokens and the
output rows are unpermuted host-side. This cuts S, exp and PV work by ~7/16
and sends x once.

All matmul operands are bf16 (PSUM accumulation stays fp32); biases and the
exp mask bias are fp32. Device pipeline per (q-block, head-pair): S^T = K@Q^T
per k-tile in PSUM -> exp with per-partition bias + 1/8 scale in one ACT op
over [128, 1024] (2 heads) -> accumulating P^T@V matmuls with a per-head ones
column producing the softmax denominator in row 64 -> reciprocal straight
from PSUM + gpsimd partition-broadcast + multiply into the attnT tile ->
proj matmuls with bias added during DVE eviction. QKV/V/proj matmul groups
stream as fillers inside the attention iterations.
"""

import numpy as np
from contextlib import ExitStack

import concourse.bass as bass
import concourse.tile as tile
from concourse import bacc, mybir
from concourse.alu_op_type import AluOpType
from concourse.bass_utils import run_bass_kernel_spmd

N_CORES = 8
B, N, DIM = 4, 2048, 768
NHEADS, HD = 12, 64
HPC = 6              # heads per core
DPC = HPC * HD       # 384 channels per core
CC = DIM // 128      # 6 contraction chunks of 128
NT_K_COMPACT = 9     # compacted key tiles (covers up to 1152 unmasked keys)
QB = 512             # q block width
NQB = N // QB        # 4 q blocks
KB = 384             # k-gen eviction block width
VW = HPC * (HD + 1)  # V width incl per-head ones column (390)
BIG = 80.0
A_SCH = 184.6650171421945   # 2^7/ln2: bf16 Schraudolph slope
B_SCH = 127 * 128 - 6       # offset, C=6 minimizes max rel err
F32 = mybir.dt.float32
BF16 = mybir.dt.bfloat16
I16 = mybir.dt.int16
AF = mybir.ActivationFunctionType
NPBF16 = mybir.dt.np(BF16)

_CACHE = {}


def build(nt_k=NT_K_COMPACT, reps=1, unroll=1):
    NK = nt_k * 128          # compacted key width
    NKB = NK // KB           # k-gen blocks
    assert NK % KB == 0

    nc = bacc.Bacc("TRN2", target_bir_lowering=False, debug=False,
                   num_devices=N_CORES)

    xT = nc.dram_tensor("xT", [DIM, N], BF16, kind="ExternalInput").ap()
    wqT = nc.dram_tensor("wqT", [DIM, DPC], BF16, kind="ExternalInput").ap()
    wkT = nc.dram_tensor("wkT", [DIM, DPC], BF16, kind="ExternalInput").ap()
    wvT = nc.dram_tensor("wvT", [DIM, VW], BF16, kind="ExternalInput").ap()
    qbias = nc.dram_tensor("qbias", [128, 3], F32, kind="ExternalInput").ap()
    mbias = nc.dram_tensor("mbias", [128, nt_k], F32, kind="ExternalInput").ap()
    mbias2 = nc.dram_tensor("mbias2", [128, nt_k], F32, kind="ExternalInput").ap()
    pwT = nc.dram_tensor("pwT", [DPC, DIM], BF16, kind="ExternalInput").ap()
    pbb = nc.dram_tensor("pbb", [128, DIM], F32, kind="ExternalInput").ap()
    vbb = nc.dram_tensor("vbb", [128, VW], F32, kind="ExternalInput").ap()
    out = nc.dram_tensor("out", [N, DIM], BF16, kind="ExternalOutput").ap()

    with tile.TileContext(nc) as tc, ExitStack() as ctx:
        # PSUM (8 banks): tag "s" [128,1024]x2 = 4, tag "o" [65,512]x2 = 2,
        # tag "pj" [128,<=512]x2 = 2 (QKV/V/proj rotation).
        psum = ctx.enter_context(tc.tile_pool(name="psum", bufs=2, space="PSUM"))
        persist = ctx.enter_context(tc.tile_pool(name="persist", bufs=1))
        qt_sb = [persist.tile([128, N], BF16, tag=f"qt{j}", name=f"qt{j}")
                 for j in range(3)]
        # K stored twice, zero-padded to 128 contraction rows per head, so S
        # matmuls stay in 128-row array mode (no 64<->128 mode-switch drains
        # against the 128-mode PV/QKV/proj stream): ktp[j][0] has head 2j in
        # rows 0:64 and zeros in 64:128, ktp[j][1] the reverse.
        ktp = [[persist.tile([128, NK], BF16, tag=f"ktp{j}_{h}",
                             name=f"ktp{j}_{h}") for h in range(2)]
               for j in range(3)]
        v_sb = [persist.tile([128, VW], BF16, tag=f"v{t}", name=f"v{t}")
                for t in range(nt_k)]
        consts = ctx.enter_context(tc.tile_pool(name="consts", bufs=1))
        qb_sb = consts.tile([128, 3], F32, tag="qb", name="qb")
        mb_sb = consts.tile([128, nt_k], F32, tag="mb", name="mb")
        mb2_sb = consts.tile([128, nt_k], F32, tag="mb2", name="mb2")
        pbb_sb = consts.tile([128, DIM], F32, tag="pbb", name="pbb")
        vbb_sb = consts.tile([128, VW], F32, tag="vbb", name="vbb")
        pw_sb = [consts.tile([128, DIM], BF16, tag=f"pw{j}", name=f"pw{j}")
                 for j in range(3)]
        xw = ctx.enter_context(tc.tile_pool(name="xw", bufs=1))
        x_sb = [xw.tile([128, N], BF16, tag=f"x{c}", name=f"x{c}")
                for c in range(CC)]
        wq_sb = [xw.tile([128, DPC], BF16, tag=f"wq{c}", name=f"wq{c}")
                 for c in range(CC)]
        wk_sb = [xw.tile([128, DPC], BF16, tag=f"wk{c}", name=f"wk{c}")
                 for c in range(CC)]
        wv_sb = [xw.tile([128, VW], BF16, tag=f"wv{c}", name=f"wv{c}")
                 for c in range(CC)]
        ppool = ctx.enter_context(tc.tile_pool(name="ppool", bufs=6))
        atpool = ctx.enter_context(tc.tile_pool(name="atpool", bufs=2))
        dpool = ctx.enter_context(tc.tile_pool(name="dpool", bufs=2))
        opool = ctx.enter_context(tc.tile_pool(name="opool", bufs=2))

        # ---- input DMAs. Tiny consts first (FIFO queue: they'd otherwise
        # wait behind megabytes), then the K-region columns of each x chunk
        # (all the prologue needs), then the q-only columns and late consts.
        nc.sync.dma_start(qb_sb[:], qbias)
        nc.sync.dma_start(mb_sb[:], mbias)
        nc.sync.dma_start(mb2_sb[:], mbias2)
        nc.sync.dma_start(vbb_sb[:], vbb)
        for c in range(CC):
            csl = slice(c * 128, (c + 1) * 128)
            nc.sync.dma_start(x_sb[c][:, 0:NK], xT[csl, 0:NK])
            nc.sync.dma_start(wk_sb[c][:], wkT[csl, :])
            nc.sync.dma_start(wq_sb[c][:], wqT[csl, :])
            nc.sync.dma_start(wv_sb[c][:], wvT[csl, :])
        for c in range(CC):
            nc.sync.dma_start(x_sb[c][:, NK:N], xT[c * 128:(c + 1) * 128, NK:N])
        nc.sync.dma_start(pbb_sb[:], pbb)
        for j in range(3):
            nc.sync.dma_start(pw_sb[j][:], pwT[j * 128:(j + 1) * 128, :])

        _q_open = {}

        def emit_q_half(j, nb, half):
            # half of a [128, 512] Q chunk (3 contraction steps each)
            dsl = bass.ts(j, 128)
            nsl = bass.ts(nb, QB)
            if half == 0:
                ps = psum.tile([128, QB], F32, tag="pj", name=f"q{j}_{nb}")
                _q_open[(j, nb)] = ps
            else:
                ps = _q_open.pop((j, nb))
            for c in range(3 * half, 3 * half + 3):
                nc.tensor.matmul(ps[:], wq_sb[c][:, dsl], x_sb[c][:, nsl],
                                 start=(c == 0), stop=(c == CC - 1))
            if half == 1:
                nc.vector.tensor_scalar(qt_sb[j][:, nsl], ps[:],
                                        qb_sb[:, j:j + 1], None,
                                        op0=AluOpType.add)

        def emit_q_group(j, nb):
            emit_q_half(j, nb, 0)
            emit_q_half(j, nb, 1)

        # zero the dead halves of the padded K tiles once
        for j in range(3):
            nc.vector.memset(ktp[j][0][64:128, :], 0)
            nc.vector.memset(ktp[j][1][0:64, :], 0)

        def emit_k_group(j, kb):
            # [128, KB] K chunk, all 6 contraction steps
            dsl = bass.ts(j, 128)
            nsl = bass.ts(kb, KB)
            ps = psum.tile([128, KB], F32, tag="pj", name=f"k{j}_{kb}")
            for c in range(CC):
                nc.tensor.matmul(ps[:], wk_sb[c][:, dsl], x_sb[c][:, nsl],
                                 start=(c == 0), stop=(c == CC - 1))
            nc.vector.tensor_copy(ktp[j][0][0:64, nsl], ps[0:64, :])
            nc.vector.tensor_copy(ktp[j][1][64:128, nsl], ps[64:128, :])

        def emit_v(t):
            tsl = bass.ts(t, 128)
            psv = psum.tile([128, VW], F32, tag="pj", name=f"v{t}")
            for c in range(CC):
                nc.tensor.matmul(psv[:], x_sb[c][:, tsl], wv_sb[c][:],
                                 start=(c == 0), stop=(c == CC - 1))
            nc.vector.tensor_tensor(v_sb[t][:], psv[:], vbb_sb[:],
                                    op=AluOpType.add)

        _pending_div = []

        def emit_attn(qb, j, fillers=(), dve_kts=()):
            # heads 2j, 2j+1 over q block qb: S^T -> exp -> P^T V -> divide
            fillers = list(fillers)
            qsl = bass.ds(qb * QB, QB)
            at_t = atpool.tile([128, QB], BF16, tag=f"at{j}", name=f"at{qb}_{j}")
            o_ps = [psum.tile([HD + 1, QB], F32, tag="o", name=f"o{qb}_{j}_{i}")
                    for i in range(2)]
            s_tiles = {}

            def emit_s(kt):
                ksl = bass.ts(kt, 128)
                s = psum.tile([128, 2 * QB], F32, tag="s",
                              name=f"s{qb}_{j}_{kt}")
                for hh in range(2):
                    nc.tensor.matmul(s[:, bass.ts(hh, QB)],
                                     ktp[j][hh][:, ksl], qt_sb[j][:, qsl],
                                     start=True, stop=True)
                s_tiles[kt] = s

            emit_s(0)
            for d in _pending_div:
                d()
            _pending_div.clear()
            for kt in range(nt_k):
                if kt + 1 < nt_k:
                    emit_s(kt + 1)   # one iteration ahead keeps ACT fed
                s = s_tiles.pop(kt)
                p = ppool.tile([128, 2 * QB], BF16, tag="p",
                               name=f"p{qb}_{j}_{kt}")
                if kt in dve_kts:
                    # Schraudolph bf16 fast-exp on DVE (max rel 3.5%):
                    # bf16 bits = round(A*(s/8 + mb) + B)
                    nc.vector.tensor_scalar(p[:].bitcast(I16), s[:],
                                            float(A_SCH * 0.125),
                                            mb2_sb[:, kt:kt + 1],
                                            op0=AluOpType.mult,
                                            op1=AluOpType.add)
                else:
                    nc.scalar.activation(p[:], s[:], AF.Exp,
                                         bias=mb_sb[:, kt:kt + 1], scale=0.125)
                for hh in range(2):
                    vsl = bass.ds((2 * j + hh) * (HD + 1), HD + 1)
                    nc.tensor.matmul(o_ps[hh], v_sb[kt][:, vsl],
                                     p[:, bass.ts(hh, QB)],
                                     start=(kt == 0), stop=(kt == nt_k - 1))
                if fillers:
                    fillers.pop(0)()
            for f in fillers:
                f()

            def _division():
                for hh in range(2):
                    rd = dpool.tile([1, QB], F32, tag="rd", name="rd")
                    bc = dpool.tile([64, QB], F32, tag="bc", name="bc")
                    nc.vector.reciprocal(rd[:], o_ps[hh][HD:HD + 1, :])
                    nc.gpsimd.partition_broadcast(bc[:], rd[:])
                    nc.vector.tensor_tensor(at_t[bass.ds(hh * 64, 64), :],
                                            o_ps[hh][0:HD, :], bc[:],
                                            op=AluOpType.mult)
            _pending_div.append(_division)
            return at_t

        def emit_proj_tile(t, qb, at_tiles):
            tsl = bass.ts(t, 128)
            po = [psum.tile([128, 384], F32, tag="pj", name=f"po{t}_{e}")
                  for e in range(2)]
            for e in range(2):
                esl = bass.ts(e, 384)
                for j in range(3):
                    nc.tensor.matmul(po[e],
                                     at_tiles[j][:, bass.ts(t - 4 * qb, 128)],
                                     pw_sb[j][:, esl],
                                     start=(j == 0), stop=(j == 2))
            for e in range(2):
                ot = opool.tile([128, 384], BF16, tag="ot", name=f"ot{t}_{e}")
                nc.vector.tensor_tensor(ot[:], po[e],
                                        pbb_sb[:, bass.ts(e, 384)],
                                        op=AluOpType.add)
                nc.sync.dma_start(out[tsl, bass.ts(e, 384)], ot[:])

        # ---- emission: stream QKV/proj groups into the attention iterations.
        # Fillers spread so later (ACT-bound) calls each get ~2 PE units;
        # proj(qb) runs up to two q-blocks late (atpool bufs=2 keeps the attnT
        # tiles of two generations alive).
        import functools

        def P(f, *a):
            return functools.partial(f, *a)

        n_iter = (reps + unroll - 1) // unroll
        rep_ctx = tc.For_i(0, n_iter, 1) if n_iter > 1 else None
        if rep_ctx is not None:
            rep_ctx.__enter__()
        for _rep in range(unroll if reps > 1 else 1):
            # minimal serial prologue: K0 all blocks, Q0 block 0, V0/V1
            for kb in range(NKB):
                emit_k_group(0, kb)
            emit_q_group(0, 0)
            emit_v(0)
            emit_v(1)
            at_q = [None] * NQB
            fills = {
                (0, 0): [P(emit_v, t) for t in range(2, nt_k)] +
                        [P(emit_k_group, 1, kb) for kb in range(NKB - 1)],
                (0, 1): [P(emit_k_group, 2, kb) for kb in range(NKB)] +
                        [P(emit_q_group, 0, 1)],
                (0, 2): [P(emit_q_group, 1, 1), P(emit_q_group, 2, 1)],
            }

            def pj(qb, t):
                return P(emit_proj_tile, t, qb, at_q[qb])

            ats = [emit_attn(0, 0, fills[(0, 0)])]
            emit_k_group(1, NKB - 1)
            emit_q_group(1, 0)
            ats.append(emit_attn(0, 1, fills[(0, 1)]))
            emit_q_group(2, 0)
            ats.append(emit_attn(0, 2, fills[(0, 2)]))
            at_q[0] = ats

            ats = [emit_attn(1, 0, [pj(0, 0), P(emit_q_group, 0, 2)]),
                   emit_attn(1, 1, [pj(0, 1), P(emit_q_group, 1, 2)]),
                   emit_attn(1, 2, [pj(0, 2), P(emit_q_group, 2, 2)])]
            at_q[1] = ats
            ats = [emit_attn(2, 0, [pj(0, 3), P(emit_q_group, 0, 3)]),
                   emit_attn(2, 1, [pj(1, 4), P(emit_q_group, 1, 3)]),
                   emit_attn(2, 2, [pj(1, 5), P(emit_q_group, 2, 3)])]
            at_q[2] = ats
            ats = [emit_attn(3, 0, [pj(1, 6), pj(1, 7)]),
                   emit_attn(3, 1, [pj(2, 8), pj(2, 9)]),
                   emit_attn(3, 2, [pj(2, 10), pj(2, 11)])]
            at_q[3] = ats
            for d in _pending_div:
                d()
            _pending_div.clear()
            for t in range(12, 16):
                emit_proj_tile(t, 3, at_q[3])
        if rep_ctx is not None:
            rep_ctx.__exit__(None, None, None)

    nc.compile()
    return nc


def _prep_inputs(x, attention_mask, qkv_w, q_bias, v_bias, proj_w, proj_b,
                 nt_k=NT_K_COMPACT):
    NK = nt_k * 128
    in_maps = []
    perms = []
    f32 = np.float32
    x = np.asarray(x, f32)
    attention_mask = np.asarray(attention_mask)
    qkv_w = np.asarray(qkv_w, f32)
    for c in range(N_CORES):
        b, h0 = c // 2, (c % 2) * HPC
        rs = slice(h0 * HD, h0 * HD + DPC)
        cols = np.flatnonzero(attention_mask[b])
        m = len(cols)
        assert m <= NK
        perm = np.concatenate([cols, np.flatnonzero(attention_mask[b] == 0)])
        perms.append(perm)
        xT = np.ascontiguousarray(x[b][perm].T)
        mb = np.full(NK, -BIG, f32)
        mb[:m] = 0.0
        mb = np.ascontiguousarray(mb.reshape(nt_k, 128).T)
        mb2 = np.ascontiguousarray(
            (B_SCH + A_SCH * mb).astype(f32))

        wqT = np.ascontiguousarray(qkv_w[rs, :].T)
        wkT = np.ascontiguousarray(
            qkv_w[DIM + h0 * HD: DIM + h0 * HD + DPC, :].T)
        wvT = np.zeros((DIM, VW), f32)
        for h in range(HPC):
            wr = qkv_w[2 * DIM + (h0 + h) * HD: 2 * DIM + (h0 + h) * HD + HD, :]
            wvT[:, h * (HD + 1): h * (HD + 1) + HD] = wr.T

        qb = np.ascontiguousarray(np.asarray(q_bias, f32)[rs].reshape(3, 128).T)
        pwT = np.ascontiguousarray(np.asarray(proj_w, f32)[:, rs].T)
        pb = np.asarray(proj_b, f32) if c % 2 == 0 else np.zeros(DIM, f32)
        pbb = np.ascontiguousarray(np.broadcast_to(pb, (128, DIM)), f32)
        vb_row = np.zeros(VW, f32)
        for h in range(HPC):
            vb_row[h * (HD + 1): h * (HD + 1) + HD] = \
                np.asarray(v_bias, f32)[(h0 + h) * HD: (h0 + h + 1) * HD]
            vb_row[h * (HD + 1) + HD] = 1.0
        vbb = np.ascontiguousarray(np.broadcast_to(vb_row, (128, VW)), f32)
        in_maps.append({
            "xT": xT.astype(NPBF16),
            "wqT": wqT.astype(NPBF16), "wkT": wkT.astype(NPBF16),
            "wvT": wvT.astype(NPBF16),
            "qbias": qb.astype(f32), "mbias": mb.astype(f32),
            "mbias2": mb2,
            "pwT": pwT.astype(NPBF16),
            "pbb": pbb, "vbb": vbb,
        })
    return in_maps, perms


def kernel(x, attention_mask, qkv_w, q_bias, v_bias, proj_w, proj_b):
    counts = np.asarray(attention_mask).astype(np.int64).sum(axis=1)
    nt_k = NT_K_COMPACT if counts.max() <= NT_K_COMPACT * 128 \
        else (int(counts.max()) + 127) // 128
    key = f"nc{nt_k}"
    if key not in _CACHE:
        _CACHE[key] = build(nt_k=nt_k)
    nc = _CACHE[key]
    in_maps, perms = _prep_inputs(x, attention_mask, qkv_w, q_bias, v_bias,
                                  proj_w, proj_b, nt_k=nt_k)
    res = run_bass_kernel_spmd(nc, in_maps, core_ids=list(range(N_CORES)))
    out = np.empty((B, N, DIM), np.float32)
    for b in range(B):
        dev = res.results[2 * b]["out"].astype(np.float32) \
            + res.results[2 * b + 1]["out"].astype(np.float32)
        out[b][perms[2 * b]] = dev
    return out


if __name__ == "__main__":
    import reference
    inputs = {k: np.asarray(v) for k, v in reference.setup_inputs().items()}
    got = kernel(**inputs)
    exp = np.asarray(reference.reference(**inputs))
    err = np.abs(got - exp).max()
    rel = err / np.abs(exp).max()
    print("max abs err:", err, "rel:", rel)


# revision 10
# speedup vs baseline: 1.1836x; 1.0054x over previous
"""Trainium2 Bass kernel for multi-head attention (B=4, N=2048, DIM=768, H=12)
with host-side mask compaction via token permutation, bf16 operands.

Sharding: 8 cores; core c handles batch b = c//2 and heads h0 = 6*(c%2)..+5.
Each core emits a partial projection output (bf16, token-permuted); the host
unpermutes rows and sums the two partials per batch.

Mask compaction: the host permutes the token axis so unmasked tokens come
first (m of them, m <= NT_K*128). One x tensor serves both paths: K/V are
computed only over the first NT_K*128 permuted tokens (positions >= m are
masked tokens and get -80 exp bias, i.e. weight ~3e-35 ~ 0, matching the
reference's -inf within ~1e-30); Q runs over all 2048 permuted tokens and the
output rows are unpermuted host-side. This cuts S, exp and PV work by ~7/16
and sends x once.

All matmul operands are bf16 (PSUM accumulation stays fp32); biases and the
exp mask bias are fp32. Device pipeline per (q-block, head-pair): S^T = K@Q^T
per k-tile in PSUM -> exp with per-partition bias + 1/8 scale in one ACT op
over [128, 1024] (2 heads) -> accumulating P^T@V matmuls with a per-head ones
column producing the softmax denominator in row 64 -> reciprocal straight
from PSUM + gpsimd partition-broadcast + multiply into the attnT tile ->
proj matmuls with bias added during DVE eviction. QKV/V/proj matmul groups
stream as fillers inside the attention iterations.
"""

import numpy as np
from contextlib import ExitStack

import concourse.bass as bass
import concourse.tile as tile
from concourse import bacc, mybir
from concourse.alu_op_type import AluOpType
from concourse.bass_utils import run_bass_kernel_spmd

N_CORES = 8
B, N, DIM = 4, 2048, 768
NHEADS, HD = 12, 64
HPC = 6              # heads per core
DPC = HPC * HD       # 384 channels per core
CC = DIM // 128      # 6 contraction chunks of 128
NT_K_COMPACT = 9     # compacted key tiles (covers up to 1152 unmasked keys)
QB = 512             # q block width
NQB = N // QB        # 4 q blocks
KB = 384             # k-gen eviction block width
VW = HPC * (HD + 1)  # V width incl per-head ones column (390)
BIG = 80.0
A_SCH = 184.6650171421945   # 2^7/ln2: bf16 Schraudolph slope
B_SCH = 127 * 128 - 6       # offset, C=6 minimizes max rel err
F32 = mybir.dt.float32
BF16 = mybir.dt.bfloat16
I16 = mybir.dt.int16
AF = mybir.ActivationFunctionType
NPBF16 = mybir.dt.np(BF16)

_CACHE = {}


def build(nt_k=NT_K_COMPACT, reps=1, unroll=1):
    NK = nt_k * 128          # compacted key width
    NKB = NK // KB           # k-gen blocks
    assert NK % KB == 0

    nc = bacc.Bacc("TRN2", target_bir_lowering=False, debug=False,
                   num_devices=N_CORES)

    xT = nc.dram_tensor("xT", [DIM, N], BF16, kind="ExternalInput").ap()
    wqT = nc.dram_tensor("wqT", [DIM, DPC], BF16, kind="ExternalInput").ap()
    wkT = nc.dram_tensor("wkT", [DIM, DPC], BF16, kind="ExternalInput").ap()
    wvT = nc.dram_tensor("wvT", [DIM, VW], BF16, kind="ExternalInput").ap()
    qbias = nc.dram_tensor("qbias", [128, 3], F32, kind="ExternalInput").ap()
    mbias = nc.dram_tensor("mbias", [128, nt_k], F32, kind="ExternalInput").ap()
    mbias2 = nc.dram_tensor("mbias2", [128, nt_k], F32, kind="ExternalInput").ap()
    pwT = nc.dram_tensor("pwT", [DPC, DIM], BF16, kind="ExternalInput").ap()
    pbb = nc.dram_tensor("pbb", [128, DIM], F32, kind="ExternalInput").ap()
    vbb = nc.dram_tensor("vbb", [128, VW], F32, kind="ExternalInput").ap()
    out = nc.dram_tensor("out", [N, DIM], BF16, kind="ExternalOutput").ap()

    with tile.TileContext(nc) as tc, ExitStack() as ctx:
        # PSUM (8 banks): tag "s" [128,1024]x2 = 4, tag "o" [65,512]x2 = 2,
        # tag "pj" [128,<=512]x2 = 2 (QKV/V/proj rotation).
        psum = ctx.enter_context(tc.tile_pool(name="psum", bufs=2, space="PSUM"))
        persist = ctx.enter_context(tc.tile_pool(name="persist", bufs=1))
        qt_sb = [persist.tile([128, N], BF16, tag=f"qt{j}", name=f"qt{j}")
                 for j in range(3)]
        # K stored twice, zero-padded to 128 contraction rows per head, so S
        # matmuls stay in 128-row array mode (no 64<->128 mode-switch drains
        # against the 128-mode PV/QKV/proj stream): ktp[j][0] has head 2j in
        # rows 0:64 and zeros in 64:128, ktp[j][1] the reverse.
        ktp = [[persist.tile([128, NK], BF16, tag=f"ktp{j}_{h}",
                             name=f"ktp{j}_{h}") for h in range(2)]
               for j in range(3)]
        v_sb = [persist.tile([128, VW], BF16, tag=f"v{t}", name=f"v{t}")
                for t in range(nt_k)]
        consts = ctx.enter_context(tc.tile_pool(name="consts", bufs=1))
        qb_sb = consts.tile([128, 3], F32, tag="qb", name="qb")
        mb_sb = consts.tile([128, nt_k], F32, tag="mb", name="mb")
        mb2_sb = consts.tile([128, nt_k], F32, tag="mb2", name="mb2")
        pbb_sb = consts.tile([128, DIM], F32, tag="pbb", name="pbb")
        vbb_sb = consts.tile([128, VW], F32, tag="vbb", name="vbb")
        pw_sb = [consts.tile([128, DIM], BF16, tag=f"pw{j}", name=f"pw{j}")
                 for j in range(3)]
        xw = ctx.enter_context(tc.tile_pool(name="xw", bufs=1))
        x_sb = [xw.tile([128, N], BF16, tag=f"x{c}", name=f"x{c}")
                for c in range(CC)]
        wq_sb = [xw.tile([128, DPC], BF16, tag=f"wq{c}", name=f"wq{c}")
                 for c in range(CC)]
        wk_sb = [xw.tile([128, DPC], BF16, tag=f"wk{c}", name=f"wk{c}")
                 for c in range(CC)]
        wv_sb = [xw.tile([128, VW], BF16, tag=f"wv{c}", name=f"wv{c}")
                 for c in range(CC)]
        ppool = ctx.enter_context(tc.tile_pool(name="ppool", bufs=6))
        atpool = ctx.enter_context(tc.tile_pool(name="atpool", bufs=2))
        dpool = ctx.enter_context(tc.tile_pool(name="dpool", bufs=2))
        opool = ctx.enter_context(tc.tile_pool(name="opool", bufs=2))

        # ---- input DMAs. Tiny consts first (FIFO queue: they'd otherwise
        # wait behind megabytes), then the K-region columns of each x chunk
        # (all the prologue needs), then the q-only columns and late consts.
        nc.sync.dma_start(qb_sb[:], qbias)
        nc.sync.dma_start(mb_sb[:], mbias)
        nc.sync.dma_start(mb2_sb[:], mbias2)
        nc.sync.dma_start(vbb_sb[:], vbb)
        for c in range(CC):
            csl = slice(c * 128, (c + 1) * 128)
            nc.sync.dma_start(x_sb[c][:, 0:NK], xT[csl, 0:NK])
            nc.sync.dma_start(wk_sb[c][:], wkT[csl, :])
            nc.sync.dma_start(wq_sb[c][:], wqT[csl, :])
            nc.sync.dma_start(wv_sb[c][:], wvT[csl, :])
        for c in range(CC):
            nc.sync.dma_start(x_sb[c][:, NK:N], xT[c * 128:(c + 1) * 128, NK:N])
        nc.sync.dma_start(pbb_sb[:], pbb)
        for j in range(3):
            nc.sync.dma_start(pw_sb[j][:], pwT[j * 128:(j + 1) * 128, :])

        _q_open = {}

        def emit_q_half(j, nb, half):
            # half of a [128, 512] Q chunk (3 contraction steps each)
            dsl = bass.ts(j, 128)
            nsl = bass.ts(nb, QB)
            if half == 0:
                ps = psum.tile([128, QB], F32, tag="pj", name=f"q{j}_{nb}")
                _q_open[(j, nb)] = ps
            else:
                ps = _q_open.pop((j, nb))
            for c in range(3 * half, 3 * half + 3):
                nc.tensor.matmul(ps[:], wq_sb[c][:, dsl], x_sb[c][:, nsl],
                                 start=(c == 0), stop=(c == CC - 1))
            if half == 1:
                nc.vector.tensor_scalar(qt_sb[j][:, nsl], ps[:],
                                        qb_sb[:, j:j + 1], None,
                                        op0=AluOpType.add)

        def emit_q_group(j, nb):
            emit_q_half(j, nb, 0)
            emit_q_half(j, nb, 1)

        # zero the dead halves of the padded K tiles once
        for j in range(3):
            nc.vector.memset(ktp[j][0][64:128, :], 0)
            nc.vector.memset(ktp[j][1][0:64, :], 0)

        def emit_k_group(j, kb):
            # [128, KB] K chunk, all 6 contraction steps
            dsl = bass.ts(j, 128)
            nsl = bass.ts(kb, KB)
            ps = psum.tile([128, KB], F32, tag="pj", name=f"k{j}_{kb}")
            for c in range(CC):
                nc.tensor.matmul(ps[:], wk_sb[c][:, dsl], x_sb[c][:, nsl],
                                 start=(c == 0), stop=(c == CC - 1))
            nc.vector.tensor_copy(ktp[j][0][0:64, nsl], ps[0:64, :])
            nc.vector.tensor_copy(ktp[j][1][64:128, nsl], ps[64:128, :])

        def emit_v(t):
            tsl = bass.ts(t, 128)
            psv = psum.tile([128, VW], F32, tag="pj", name=f"v{t}")
            for c in range(CC):
                nc.tensor.matmul(psv[:], x_sb[c][:, tsl], wv_sb[c][:],
                                 start=(c == 0), stop=(c == CC - 1))
            nc.vector.tensor_tensor(v_sb[t][:], psv[:], vbb_sb[:],
                                    op=AluOpType.add)

        _pending_div = []

        def emit_attn(qb, j, fillers=(), dve_kts=()):
            # heads 2j, 2j+1 over q block qb: S^T -> exp -> P^T V -> divide
            fillers = list(fillers)
            qsl = bass.ds(qb * QB, QB)
            at_t = atpool.tile([128, QB], BF16, tag=f"at{j}", name=f"at{qb}_{j}")
            o_ps = [psum.tile([HD + 1, QB], F32, tag="o", name=f"o{qb}_{j}_{i}")
                    for i in range(2)]
            s_tiles = {}

            def emit_s(kt):
                ksl = bass.ts(kt, 128)
                s = psum.tile([128, 2 * QB], F32, tag="s",
                              name=f"s{qb}_{j}_{kt}")
                for hh in range(2):
                    nc.tensor.matmul(s[:, bass.ts(hh, QB)],
                                     ktp[j][hh][:, ksl], qt_sb[j][:, qsl],
                                     start=True, stop=True)
                s_tiles[kt] = s

            emit_s(0)
            for d in _pending_div:
                d()
            _pending_div.clear()
            for kt in range(nt_k):
                if kt + 1 < nt_k:
                    emit_s(kt + 1)   # one iteration ahead keeps ACT fed
                s = s_tiles.pop(kt)
                p = ppool.tile([128, 2 * QB], BF16, tag="p",
                               name=f"p{qb}_{j}_{kt}")
                if kt in dve_kts:
                    # Schraudolph bf16 fast-exp on DVE (max rel 3.5%):
                    # bf16 bits = round(A*(s/8 + mb) + B)
                    nc.vector.tensor_scalar(p[:].bitcast(I16), s[:],
                                            float(A_SCH * 0.125),
                                            mb2_sb[:, kt:kt + 1],
                                            op0=AluOpType.mult,
                                            op1=AluOpType.add)
                else:
                    nc.scalar.activation(p[:], s[:], AF.Exp,
                                         bias=mb_sb[:, kt:kt + 1], scale=0.125)
                for hh in range(2):
                    vsl = bass.ds((2 * j + hh) * (HD + 1), HD + 1)
                    nc.tensor.matmul(o_ps[hh], v_sb[kt][:, vsl],
                                     p[:, bass.ts(hh, QB)],
                                     start=(kt == 0), stop=(kt == nt_k - 1))
                if fillers:
                    fillers.pop(0)()
            for f in fillers:
                f()

            def _division():
                for hh in range(2):
                    rd = dpool.tile([1, QB], F32, tag="rd", name="rd")
                    bc = dpool.tile([64, QB], F32, tag="bc", name="bc")
                    nc.vector.reciprocal(rd[:], o_ps[hh][HD:HD + 1, :])
                    nc.gpsimd.partition_broadcast(bc[:], rd[:])
                    nc.vector.tensor_tensor(at_t[bass.ds(hh * 64, 64), :],
                                            o_ps[hh][0:HD, :], bc[:],
                                            op=AluOpType.mult)
            _pending_div.append(_division)
            return at_t

        def emit_proj_tile(t, qb, at_tiles):
            tsl = bass.ts(t, 128)
            po = [psum.tile([128, 384], F32, tag="pj", name=f"po{t}_{e}")
                  for e in range(2)]
            for e in range(2):
                esl = bass.ts(e, 384)
                for j in range(3):
                    nc.tensor.matmul(po[e],
                                     at_tiles[j][:, bass.ts(t - 4 * qb, 128)],
                                     pw_sb[j][:, esl],
                                     start=(j == 0), stop=(j == 2))
            for e in range(2):
                ot = opool.tile([128, 384], BF16, tag="ot", name=f"ot{t}_{e}")
                nc.vector.tensor_tensor(ot[:], po[e],
                                        pbb_sb[:, bass.ts(e, 384)],
                                        op=AluOpType.add)
                nc.sync.dma_start(out[tsl, bass.ts(e, 384)], ot[:])

        # ---- emission: stream QKV/proj groups into the attention iterations.
        # Fillers spread so later (ACT-bound) calls each get ~2 PE units;
        # proj(qb) runs up to two q-blocks late (atpool bufs=2 keeps the attnT
        # tiles of two generations alive).
        import functools

        def P(f, *a):
            return functools.partial(f, *a)

        n_iter = (reps + unroll - 1) // unroll
        rep_ctx = tc.For_i(0, n_iter, 1) if n_iter > 1 else None
        if rep_ctx is not None:
            rep_ctx.__enter__()
        for _rep in range(unroll if reps > 1 else 1):
            # minimal serial prologue: K0 all blocks, Q0 block 0, V0/V1
            for kb in range(NKB):
                emit_k_group(0, kb)
            emit_q_group(0, 0)
            emit_v(0)
            emit_v(1)
            at_q = [None] * NQB
            fills = {
                (0, 0): [P(emit_v, t) for t in range(2, nt_k)] +
                        [P(emit_k_group, 1, kb) for kb in range(NKB - 1)],
                (0, 1): [P(emit_k_group, 2, kb) for kb in range(NKB)] +
                        [P(emit_q_group, 0, 1)],
                (0, 2): [P(emit_q_group, 1, 1), P(emit_q_group, 2, 1)],
            }

            def pj(qb, t):
                return P(emit_proj_tile, t, qb, at_q[qb])

            ats = [emit_attn(0, 0, fills[(0, 0)])]
            emit_k_group(1, NKB - 1)
            emit_q_group(1, 0)
            ats.append(emit_attn(0, 1, fills[(0, 1)]))
            emit_q_group(2, 0)
            ats.append(emit_attn(0, 2, fills[(0, 2)]))
            at_q[0] = ats

            ats = [emit_attn(1, 0, [pj(0, 0), P(emit_q_group, 0, 2)]),
                   emit_attn(1, 1, [pj(0, 1), P(emit_q_group, 1, 2)]),
                   emit_attn(1, 2, [pj(0, 2), P(emit_q_group, 2, 2)])]
            at_q[1] = ats
            ats = [emit_attn(2, 0, [pj(0, 3), P(emit_q_group, 0, 3)]),
                   emit_attn(2, 1, [pj(1, 4), P(emit_q_group, 1, 3)]),
                   emit_attn(2, 2, [pj(1, 5), P(emit_q_group, 2, 3)])]
            at_q[2] = ats
            ats = [emit_attn(3, 0, [pj(1, 6), pj(1, 7)]),
                   emit_attn(3, 1, [pj(2, 8), pj(2, 9)]),
                   emit_attn(3, 2, [pj(2, 10), pj(2, 11)])]
            at_q[3] = ats
            for d in _pending_div:
                d()
            _pending_div.clear()
            for t in range(12, 16):
                emit_proj_tile(t, 3, at_q[3])
        if rep_ctx is not None:
            rep_ctx.__exit__(None, None, None)

    nc.compile()
    return nc


def _prep_inputs(x, attention_mask, qkv_w, q_bias, v_bias, proj_w, proj_b,
                 nt_k=NT_K_COMPACT):
    NK = nt_k * 128
    in_maps = []
    perms = []
    f32 = np.float32
    x = np.asarray(x, f32)
    attention_mask = np.asarray(attention_mask)
    qkv_w = np.asarray(qkv_w, f32)
    for c in range(N_CORES):
        b, h0 = c // 2, (c % 2) * HPC
        rs = slice(h0 * HD, h0 * HD + DPC)
        cols = np.flatnonzero(attention_mask[b])
        m = len(cols)
        assert m <= NK
        perm = np.concatenate([cols, np.flatnonzero(attention_mask[b] == 0)])
        perms.append(perm)
        xT = np.ascontiguousarray(x[b][perm].T)
        mb = np.full(NK, -BIG, f32)
        mb[:m] = 0.0
        mb = np.ascontiguousarray(mb.reshape(nt_k, 128).T)
        mb2 = np.ascontiguousarray(
            (B_SCH + A_SCH * mb).astype(f32))

        wqT = np.ascontiguousarray(qkv_w[rs, :].T)
        wkT = np.ascontiguousarray(
            qkv_w[DIM + h0 * HD: DIM + h0 * HD + DPC, :].T)
        wvT = np.zeros((DIM, VW), f32)
        for h in range(HPC):
            wr = qkv_w[2 * DIM + (h0 + h) * HD: 2 * DIM + (h0 + h) * HD + HD, :]
            wvT[:, h * (HD + 1): h * (HD + 1) + HD] = wr.T

        qb = np.ascontiguousarray(np.asarray(q_bias, f32)[rs].reshape(3, 128).T)
        pwT = np.ascontiguousarray(np.asarray(proj_w, f32)[:, rs].T)
        pb = np.asarray(proj_b, f32) if c % 2 == 0 else np.zeros(DIM, f32)
        pbb = np.ascontiguousarray(np.broadcast_to(pb, (128, DIM)), f32)
        vb_row = np.zeros(VW, f32)
        for h in range(HPC):
            vb_row[h * (HD + 1): h * (HD + 1) + HD] = \
                np.asarray(v_bias, f32)[(h0 + h) * HD: (h0 + h + 1) * HD]
            vb_row[h * (HD + 1) + HD] = 1.0
        vbb = np.ascontiguousarray(np.broadcast_to(vb_row, (128, VW)), f32)
        in_maps.append({
            "xT": xT.astype(NPBF16),
            "wqT": wqT.astype(NPBF16), "wkT": wkT.astype(NPBF16),
            "wvT": wvT.astype(NPBF16),
            "qbias": qb.astype(f32), "mbias": mb.astype(f32),
            "mbias2": mb2,
            "pwT": pwT.astype(NPBF16),
            "pbb": pbb, "vbb": vbb,
        })
    return in_maps, perms


def kernel(x, attention_mask, qkv_w, q_bias, v_bias, proj_w, proj_b):
    counts = np.asarray(attention_mask).astype(np.int64).sum(axis=1)
    if counts.max() <= NT_K_COMPACT * 128:
        nt_k = NT_K_COMPACT
    else:
        nt_k = (int(counts.max()) + 127) // 128
        nt_k += (-nt_k) % 3          # k-gen blocks are 384 wide

    key = f"nc{nt_k}"
    if key not in _CACHE:
        _CACHE[key] = build(nt_k=nt_k)
    nc = _CACHE[key]
    in_maps, perms = _prep_inputs(x, attention_mask, qkv_w, q_bias, v_bias,
                                  proj_w, proj_b, nt_k=nt_k)
    res = run_bass_kernel_spmd(nc, in_maps, core_ids=list(range(N_CORES)))
    out = np.empty((B, N, DIM), np.float32)
    for b in range(B):
        dev = res.results[2 * b]["out"].astype(np.float32) \
            + res.results[2 * b + 1]["out"].astype(np.float32)
        out[b][perms[2 * b]] = dev
    return out


if __name__ == "__main__":
    import reference
    inputs = {k: np.asarray(v) for k, v in reference.setup_inputs().items()}
    got = kernel(**inputs)
    exp = np.asarray(reference.reference(**inputs))
    err = np.abs(got - exp).max()
    rel = err / np.abs(exp).max()
    print("max abs err:", err, "rel:", rel)
